# revision 1
# baseline (speedup 1.0000x reference)
import numpy as np
import jax
import jax.numpy as jnp
from functools import partial

# nn_AttentionPoolingLayer: hardcoded problem shapes (see spec)
B, T, D = 2048, 200, 64
M = 8  # NeuronCores; pure data parallel over batch, weights replicated


def _prelu(x, alpha):
    return jnp.maximum(x, 0) + alpha * jnp.minimum(x, 0)


@partial(jax.pmap, axis_name="shard")
def _fwd(q, k, W1, b1, a1, W2, b2, a2, W3, b3, a3, Wl, bl):
    # q: [b,1,D] broadcast over T; k: [b,T,D]
    qt = jnp.broadcast_to(q, k.shape)
    att_in = jnp.concatenate([qt, k, qt - k, qt * k], axis=-1)  # [b,T,4D]
    h = _prelu(jnp.einsum("btf,fh->bth", att_in, W1) + b1, a1)
    h = _prelu(jnp.einsum("btf,fh->bth", h, W2) + b2, a2)
    h = _prelu(jnp.einsum("btf,fh->bth", h, W3) + b3, a3)
    score = (jnp.einsum("btf,fo->bto", h, Wl) + bl)[..., 0]  # [b,T]
    mask = k[:, :, 0] != 0
    score = jnp.where(mask, score, 0.0)
    return jnp.einsum("bt,btd->bd", score, k)  # [b,D]


def kernel(q, k, W1, b1, a1, W2, b2, a2, W3, b3, a3, Wl, bl):
    q = np.asarray(q, dtype=np.float32)
    k = np.asarray(k, dtype=np.float32)
    Bfull = q.shape[0]
    bs = Bfull // M

    qs = np.ascontiguousarray(q.reshape(M, bs, 1, q.shape[-1]))
    ks = np.ascontiguousarray(k.reshape(M, bs, k.shape[1], k.shape[2]))

    def rep(w):
        w = np.asarray(w, dtype=np.float32)
        return np.ascontiguousarray(np.broadcast_to(w, (M,) + w.shape))

    out = _fwd(
        qs, ks,
        rep(W1), rep(b1), rep(a1),
        rep(W2), rep(b2), rep(a2),
        rep(W3), rep(b3), rep(a3),
        rep(Wl), rep(bl),
    )
    out = np.asarray(jax.device_get(out), dtype=np.float32)
    return out.reshape(Bfull, out.shape[-1])
